# revision 23
# baseline (speedup 1.0000x reference)
"""Causal multi-head attention kernel for 8 Trainium2 NeuronCores.

Problem: x(4,2048,512) -> qkv proj -> 8-head causal attention -> out proj.
Sharding: core c handles batch b=c//2, heads 4*(c%2)..4*(c%2)+3.
Each core returns a partial (2048,512) output (its 4 heads' contribution
through w_out); host sums the two cores of each batch and adds b_out.

Per-core device algorithm (all fp32 data, fp32r matmuls):
  P1  QKV^T projection from host-pretransposed xT (512,2048):
        QT/KT stored (128=2heads, 2048) per pair; V stored natural
        (s,65-per-head) with a ones column (from an augmented weight
        matrix with a bias row) so the PV matmul also produces softmax
        denominators.
  P2  per head: S_T = K Q^T (k on partitions, q free), exp via ACT
        (scale=1/8 folded in, no max subtraction -- scores are O(6)),
        causal diagonal masked by elementwise multiply, then
        out'/denom = [V|1]^T @ P~^T accumulated in PSUM.
  P3  reciprocal of denoms, broadcast across partitions via a tiny
        matmul, normalize out', final projection out = O~ @ w_out_slice.
"""

import os
import sys

import numpy as np

if "/opt/trn_rl_repo" not in sys.path:
    sys.path.insert(0, "/opt/trn_rl_repo")

import concourse.bass as bass
import concourse.mybir as mybir
import concourse.tile as tile
from concourse import bacc
from concourse.bass_utils import run_bass_kernel_spmd

F32 = mybir.dt.float32
F32R = mybir.dt.float32r
AF = mybir.ActivationFunctionType

S = 2048
D = 512
HD = 64
HPC = 4          # heads per core
NCORES = 8
SCALE = 0.125    # 1/sqrt(64)
VW = HD + 1      # 65: V plus ones column

# column offsets inside the packed (128, FTOT) input
OFF_XT = 0                      # 4 tiles of (128, 2048)
OFF_WQ = OFF_XT + 4 * 2048      # 4 tiles of (128, 256)
OFF_WK = OFF_WQ + 4 * 256
OFF_WVA = OFF_WK + 4 * 256      # 4 tiles of (128, 260)
OFF_WVL = OFF_WVA + 4 * (HPC * VW)   # (128, 260), row 0 = bias/ones row
OFF_WO = OFF_WVL + HPC * VW     # 2 tiles of (128, 512)
OFF_BQ = OFF_WO + 2 * 512       # row 0: bq (256)
OFF_BK = OFF_BQ + 256           # row 0: bk (256)
OFF_ONES = OFF_BK + 256         # row 0: ones (2048)
FTOT = OFF_ONES + S


def build_nc():
    nc = bacc.Bacc("TRN2", target_bir_lowering=False, debug=False)

    wpack = nc.dram_tensor("wpack", [128, FTOT], F32, kind="ExternalInput").ap()
    maskmm = nc.dram_tensor("maskmm", [128, 256], mybir.dt.bfloat16,
                            kind="ExternalInput").ap()
    out = nc.dram_tensor("out", [S, D], F32, kind="ExternalOutput").ap()

    with tile.TileContext(nc) as tc:
        _build_kernel(tc, wpack, maskmm, out)
    nc.compile()
    return nc


def _build_kernel(tc, wpack, maskmm, out):
    nc = tc.nc
    from contextlib import ExitStack

    ctx = ExitStack()
    with ctx:
        pers = ctx.enter_context(tc.tile_pool(name="pers", bufs=1))
        stg_cm = tc.tile_pool(name="stg", bufs=1)
        stg = stg_cm.__enter__()
        ppsum = ctx.enter_context(
            tc.tile_pool(name="ppsum", bufs=2, space="PSUM"))   # P1/P3 matmuls
        spsum = ctx.enter_context(
            tc.tile_pool(name="spsum", bufs=1, space="PSUM"))   # scores (4 banks)
        opsum = ctx.enter_context(
            tc.tile_pool(name="opsum", bufs=2, space="PSUM"))   # PV accumulator

        # ---------- P0: one packed DMA + one f32r rounding copy ----------
        raw = stg.tile([128, FTOT], F32, tag="stage", name="stg_wpack")
        nc.sync.dma_start(raw[:], wpack[:])
        wr = pers.tile([128, FTOT], F32R, tag="wr", name="wr")
        nc.vector.tensor_copy(wr[:], raw[:])

        xT_sb = [wr[:, OFF_XT + S * dc:OFF_XT + S * (dc + 1)]
                 for dc in range(4)]
        wq_sb = [wr[:, OFF_WQ + 256 * dc:OFF_WQ + 256 * (dc + 1)]
                 for dc in range(4)]
        wk_sb = [wr[:, OFF_WK + 256 * dc:OFF_WK + 256 * (dc + 1)]
                 for dc in range(4)]
        wva_sb = [wr[:, OFF_WVA + HPC * VW * dc:OFF_WVA + HPC * VW * (dc + 1)]
                  for dc in range(4)]
        wva_last = wr[0:1, OFF_WVL:OFF_WVL + HPC * VW]
        wo_sb = [wr[:, OFF_WO + D * p:OFF_WO + D * (p + 1)]
                 for p in range(2)]
        bq_row = wr[0:1, OFF_BQ:OFF_BQ + 256]
        bk_row = wr[0:1, OFF_BK:OFF_BK + 256]

        ones_row = wr[0:1, OFF_ONES:OFF_ONES + S]
        ones64 = wr[0:1, OFF_ONES:OFF_ONES + 64]
        mkmm = pers.tile([128, 256], mybir.dt.bfloat16, tag="mkmm", name="mkmm")
        nc.sync.dma_start(mkmm[:], maskmm[:])
        mm_su = mkmm[:, 0:128]     # strict-upper ones
        mm_ni = mkmm[:, 128:256]   # -1e5 * I

        stg_cm.__exit__(None, None, None)
        ptp = ctx.enter_context(tc.tile_pool(name="ptp", bufs=3))  # exp(S_T)
        outp = ctx.enter_context(tc.tile_pool(name="outp", bufs=2))

        # ---------- P1a: QT / KT per pair ----------
        QT, KT = [], []
        for p in range(2):
            qt = pers.tile([128, S], F32R, tag=f"QT{p}", name=f"QT{p}")
            kt = pers.tile([128, S], F32R, tag=f"KT{p}", name=f"KT{p}")
            QT.append(qt)
            KT.append(kt)
            for (w_sb, b_row, dst) in ((wq_sb, bq_row, qt), (wk_sb, bk_row, kt)):
                for sc in range(4):
                    ps = ppsum.tile([128, 512], F32, tag="pp", name="p1ps")
                    for dc in range(4):
                        nc.tensor.matmul(
                            ps[:],
                            w_sb[dc][:, 128 * p:128 * (p + 1)],
                            xT_sb[dc][:, 512 * sc:512 * (sc + 1)],
                            start=(dc == 0), stop=False)
                    nc.tensor.matmul(
                        ps[:],
                        b_row[:, 128 * p:128 * (p + 1)],
                        ones_row[:, 512 * sc:512 * (sc + 1)],
                        start=False, stop=True)
                    nc.vector.tensor_copy(
                        dst[:, 512 * sc:512 * (sc + 1)], ps[:])

        # ---------- P1b: V augmented (natural layout) ----------
        vaug = pers.tile([128, 16 * HPC * VW], F32R, tag="vaug", name="vaug")
        for st in range(16):
            ps = ppsum.tile([128, HPC * VW], F32, tag="pp", name="p1vps")
            for dc in range(4):
                nc.tensor.matmul(
                    ps[:],
                    xT_sb[dc][:, 128 * st:128 * (st + 1)],
                    wva_sb[dc][:],
                    start=(dc == 0), stop=False)
            nc.tensor.matmul(
                ps[:],
                ones_row[:, 128 * st:128 * (st + 1)],
                wva_last[:],
                start=False, stop=True)
            nc.vector.tensor_copy(
                vaug[:, (HPC * VW) * st:(HPC * VW) * (st + 1)], ps[:])

        # ---------- P2: attention per head ----------
        OT, OTN = [], []
        for p in range(2):
            ot = pers.tile([128, S], F32, tag=f"OT{p}", name=f"OT{p}")
            OT.append(ot)
            otn = pers.tile([128, S], F32R, tag=f"OTN{p}", name=f"OTN{p}")
            OTN.append(otn)
        rcp = ctx.enter_context(tc.tile_pool(name="rcp", bufs=2))

        # memset the scores psum buffer once (exp may read lanes no matmul
        # wrote this iteration; stale-but-bounded is fine, uninit is not)
        ps_s_init = spsum.tile([128, 2048], F32, tag="ps_s", name="ps_s")
        nc.vector.memset(ps_s_init[:], 0.0)

        for h in range(HPC):
            p, sub = h // 2, h % 2
            qrows = slice(64 * sub, 64 * sub + 64)
            for qq in range(4):
                ps_o = opsum.tile([VW, 512], F32, tag="ps_o", name="ps_o")
                for g in range(qq + 1):
                    ps_s = spsum.tile([128, 2048], F32, tag="ps_s", name="ps_s")
                    for j in range(4):
                        kk = 4 * g + j
                        so = max(kk * 128 - qq * 512, 0)
                        diag = (g == qq)
                        nc.tensor.matmul(
                            ps_s[:, 512 * j + so:512 * (j + 1)],
                            KT[p][qrows, 128 * kk:128 * (kk + 1)],
                            QT[p][qrows, 512 * qq + so:512 * (qq + 1)],
                            start=True, stop=not diag)
                        if diag:
                            # += -1e5 where k > q on the 128-wide diag block
                            nc.tensor.matmul(
                                ps_s[:, 512 * j + so:512 * j + so + 128],
                                mm_su, mm_ni,
                                start=False, stop=True,
                                skip_group_check=True)
                    pt = ptp.tile([128, 2048], F32R, tag="pt", name="pt")
                    nc.scalar.activation(pt[:], ps_s[:], AF.Exp, scale=SCALE)
                    for j in range(4):
                        kk = 4 * g + j
                        so = max(kk * 128 - qq * 512, 0)
                        nc.tensor.matmul(
                            ps_o[:, so:512],
                            vaug[:, (HPC * VW) * kk + VW * h:
                                 (HPC * VW) * kk + VW * h + VW],
                            pt[:, 512 * j + so:512 * (j + 1)],
                            start=(kk == 0), stop=(kk == 4 * qq + 3))
                nc.vector.tensor_copy(
                    OT[p][qrows, 512 * qq:512 * (qq + 1)], ps_o[0:64, :])
                rr = rcp.tile([1, 512], F32R, tag="rr", name="rr")
                with nc.allow_low_precision(
                        reason="f32r rounding of softmax recip"):
                    nc.vector.reciprocal(rr[:], ps_o[64:65, :])
                ps_b = ppsum.tile([64, 512], F32, tag="pp", name="p3bps")
                nc.tensor.matmul(ps_b[:], ones64[:], rr[:],
                                 start=True, stop=True)
                nc.vector.tensor_mul(
                    OTN[p][qrows, 512 * qq:512 * (qq + 1)],
                    OT[p][qrows, 512 * qq:512 * (qq + 1)],
                    ps_b[:])

        # ---------- P3: output projection ----------
        for gidx in range(2):
            osb = outp.tile([128, 8 * D], F32, tag="osb", name="osb")
            for u in range(8):
                t = 8 * gidx + u
                ps_f = ppsum.tile([128, 512], F32, tag="pp", name="p3fps")
                for p in range(2):
                    nc.tensor.matmul(
                        ps_f[:],
                        OTN[p][:, 128 * t:128 * (t + 1)],
                        wo_sb[p][:],
                        start=(p == 0), stop=(p == 1))
                nc.vector.tensor_copy(osb[:, D * u:D * (u + 1)], ps_f[:])
            out_view = out[1024 * gidx:1024 * (gidx + 1), :].rearrange(
                "(u p) c -> p u c", p=128)
            nc.sync.dma_start(out_view, osb[:].rearrange(
                "p (u c) -> p u c", u=8))


def make_in_maps(x, w_qkv, b_qkv, w_out, b_out):
    x = np.asarray(x, dtype=np.float32)
    w_qkv = np.asarray(w_qkv, dtype=np.float32)
    b_qkv = np.asarray(b_qkv, dtype=np.float32)
    w_out = np.asarray(w_out, dtype=np.float32)

    wr = w_qkv.reshape(D, 3, 8, HD)
    br = b_qkv.reshape(3, 8, HD)
    import ml_dtypes
    maskmm = np.zeros((128, 256), dtype=ml_dtypes.bfloat16)
    maskmm[:, 0:128] = np.triu(np.ones((128, 128)), k=1).astype(
        ml_dtypes.bfloat16)
    maskmm[:, 128:256] = (np.eye(128) * -1e5).astype(ml_dtypes.bfloat16)

    in_maps = []
    for c in range(NCORES):
        b = c // 2
        h0 = 4 * (c % 2)
        xT = np.ascontiguousarray(x[b].T)                       # (512, 2048)
        wq = wr[:, 0, h0:h0 + 4].reshape(D, 256)
        wk = wr[:, 1, h0:h0 + 4].reshape(D, 256)
        wv = wr[:, 2, h0:h0 + 4].reshape(D, 256)
        bv = br[2, h0:h0 + 4].reshape(256)
        wva = np.zeros((D + 1, HPC * VW), dtype=np.float32)
        for j in range(HPC):
            wva[:D, VW * j:VW * j + HD] = wv[:, HD * j:HD * (j + 1)]
            wva[D, VW * j:VW * j + HD] = bv[HD * j:HD * (j + 1)]
            wva[D, VW * j + HD] = 1.0
        bq = br[0, h0:h0 + 4].reshape(256)
        bk = br[1, h0:h0 + 4].reshape(256)
        wo = w_out.reshape(8, HD, D)[h0:h0 + 4].reshape(256, D)

        wpack = np.zeros((128, FTOT), dtype=np.float32)
        for dc in range(4):
            wpack[:, OFF_XT + S * dc:OFF_XT + S * (dc + 1)] = \
                xT[128 * dc:128 * (dc + 1)]
            wpack[:, OFF_WQ + 256 * dc:OFF_WQ + 256 * (dc + 1)] = \
                wq[128 * dc:128 * (dc + 1)]
            wpack[:, OFF_WK + 256 * dc:OFF_WK + 256 * (dc + 1)] = \
                wk[128 * dc:128 * (dc + 1)]
            wpack[:, OFF_WVA + HPC * VW * dc:OFF_WVA + HPC * VW * (dc + 1)] = \
                wva[128 * dc:128 * (dc + 1)]
        wpack[0, OFF_WVL:OFF_WVL + HPC * VW] = wva[D]
        for p in range(2):
            wpack[:, OFF_WO + D * p:OFF_WO + D * (p + 1)] = \
                wo[128 * p:128 * (p + 1)]
        wpack[0, OFF_BQ:OFF_BQ + 256] = bq
        wpack[0, OFF_BK:OFF_BK + 256] = bk
        wpack[0, OFF_ONES:OFF_ONES + S] = 1.0

        in_maps.append({"wpack": wpack, "maskmm": maskmm})
    return in_maps


_NC_CACHE = None


def get_nc():
    global _NC_CACHE
    if _NC_CACHE is None:
        _NC_CACHE = build_nc()
    return _NC_CACHE


def run_cores(x, w_qkv, b_qkv, w_out, b_out, trace=False, trace_cores=None):
    nc = get_nc()
    in_maps = make_in_maps(x, w_qkv, b_qkv, w_out, b_out)
    br = run_bass_kernel_spmd(
        nc, in_maps, list(range(NCORES)),
        trace=trace, trace_cores=trace_cores)
    return br


def assemble(results, b_out):
    b_out = np.asarray(b_out, dtype=np.float32)
    out = np.empty((4, S, D), dtype=np.float32)
    for b in range(4):
        out[b] = results[2 * b]["out"] + results[2 * b + 1]["out"] + b_out
    return out


def kernel(x, w_qkv, b_qkv, w_out, b_out):
    br = run_cores(x, w_qkv, b_qkv, w_out, b_out, trace=False)
    return assemble(br.results, b_out)


# revision 25
# speedup vs baseline: 1.2383x; 1.2383x over previous
"""Causal multi-head attention kernel for 8 Trainium2 NeuronCores.

Problem: x(4,2048,512) -> qkv proj -> 8-head causal attention -> out proj.
Sharding: core c handles batch b=c//2, heads 4*(c%2)..4*(c%2)+3.
Each core returns a partial (2048,512) output (its 4 heads' contribution
through w_out); host sums the two cores of each batch and adds b_out.

Per-core device algorithm (bf16 matmuls, fp32 psum/softmax):
  P1  QKV^T projection from host-pretransposed xT (512,2048):
        QT/KT stored (128=2heads, 2048) per pair; V stored natural with a
        ones column (from an augmented weight matrix with a bias row) so
        the PV matmul also produces softmax denominators. Biases fold in
        as rank-1 (bias row x ones row) matmuls.
  P2  per head: S_T = K Q^T (k on partitions, q free) in fp32 PSUM,
        causal diag masked by accumulating -1e5*(k>q) via a bf16 matmul,
        exp via ACT (scale=1/8 folded, no max subtraction -- scores are
        O(7)), then out'/denom = [V|1]^T @ P~^T accumulated in PSUM.
  P3  denominators DMA-gathered into a 128-partition tile (DVE
        reciprocal is 8 cyc/elem/lane -- needs all lanes), one
        reciprocal per head-pair, DMA back to a row, broadcast across
        partitions via a ones(1,64) matmul, normalize out', project.
"""

import os
import sys

import numpy as np

if "/opt/trn_rl_repo" not in sys.path:
    sys.path.insert(0, "/opt/trn_rl_repo")

import ml_dtypes

import concourse.bass as bass
import concourse.mybir as mybir
import concourse.tile as tile
from concourse import bacc
from concourse.bass_utils import run_bass_kernel_spmd

F32 = mybir.dt.float32
BF16 = mybir.dt.bfloat16
AF = mybir.ActivationFunctionType

S = 2048
D = 512
HD = 64
HPC = 4          # heads per core
NCORES = 8
SCALE = 0.125    # 1/sqrt(64)
VW = HD + 1      # 65: V plus ones column
VWS = HPC * VW   # 260

# column offsets inside the packed bf16 (128, FTOT) input
OFF_XT = 0                      # 4 tiles of (128, 2048)
OFF_WQ = OFF_XT + 4 * S         # 4 tiles of (128, 256)
OFF_WK = OFF_WQ + 4 * 256
OFF_WVA = OFF_WK + 4 * 256      # 4 tiles of (128, 260)
OFF_WVL = OFF_WVA + 4 * VWS     # (128, 260), row 0 = bias/ones row
OFF_WO = OFF_WVL + VWS          # 2 tiles of (128, 512)
OFF_BQ = OFF_WO + 2 * D         # row 0: bq (256)
OFF_BK = OFF_BQ + 256           # row 0: bk (256)
OFF_ONES = OFF_BK + 256         # row 0: ones (2048)
OFF_SU = OFF_ONES + S           # (128,128) strict-upper ones (diag mask)
OFF_NI = OFF_SU + 128           # (128,128) -1e5 * I
FTOT = OFF_NI + 128


def build_nc():
    nc = bacc.Bacc("TRN2", target_bir_lowering=False, debug=False)

    wpack = nc.dram_tensor("wpack", [128, FTOT], BF16,
                           kind="ExternalInput").ap()
    out = nc.dram_tensor("out", [S, D], F32, kind="ExternalOutput").ap()

    with tile.TileContext(nc) as tc:
        _build_kernel(tc, wpack, out)
    nc.compile()
    return nc


def _build_kernel(tc, wpack, out):
    nc = tc.nc
    from contextlib import ExitStack

    ctx = ExitStack()
    with ctx:
        pers = ctx.enter_context(tc.tile_pool(name="pers", bufs=1))
        ppsum = ctx.enter_context(
            tc.tile_pool(name="ppsum", bufs=2, space="PSUM"))   # P1/P3
        spsum = ctx.enter_context(
            tc.tile_pool(name="spsum", bufs=1, space="PSUM"))   # scores
        opsum = ctx.enter_context(
            tc.tile_pool(name="opsum", bufs=2, space="PSUM"))   # PV accum
        ptp = ctx.enter_context(tc.tile_pool(name="ptp", bufs=3))
        outp = ctx.enter_context(tc.tile_pool(name="outp", bufs=2))
        dnp = ctx.enter_context(tc.tile_pool(name="dnp", bufs=3))

        # ---------- P0: one packed bf16 DMA ----------
        wr = pers.tile([128, FTOT], BF16, tag="wr", name="wr")
        nc.sync.dma_start(wr[:], wpack[:])

        xT_sb = [wr[:, OFF_XT + S * dc:OFF_XT + S * (dc + 1)]
                 for dc in range(4)]
        wq_sb = [wr[:, OFF_WQ + 256 * dc:OFF_WQ + 256 * (dc + 1)]
                 for dc in range(4)]
        wk_sb = [wr[:, OFF_WK + 256 * dc:OFF_WK + 256 * (dc + 1)]
                 for dc in range(4)]
        wva_sb = [wr[:, OFF_WVA + VWS * dc:OFF_WVA + VWS * (dc + 1)]
                  for dc in range(4)]
        wva_last = wr[0:1, OFF_WVL:OFF_WVL + VWS]
        wo_sb = [wr[:, OFF_WO + D * p:OFF_WO + D * (p + 1)]
                 for p in range(2)]
        bq_row = wr[0:1, OFF_BQ:OFF_BQ + 256]
        bk_row = wr[0:1, OFF_BK:OFF_BK + 256]
        ones_row = wr[0:1, OFF_ONES:OFF_ONES + S]
        ones64 = wr[0:1, OFF_ONES:OFF_ONES + 64]
        mm_su = wr[:, OFF_SU:OFF_SU + 128]
        mm_ni = wr[:, OFF_NI:OFF_NI + 128]

        # ---------- P1a: QT / KT per pair (bf16) ----------
        QT, KT = [], []
        for p in range(2):
            qt = pers.tile([128, S], BF16, tag=f"QT{p}", name=f"QT{p}")
            kt = pers.tile([128, S], BF16, tag=f"KT{p}", name=f"KT{p}")
            QT.append(qt)
            KT.append(kt)
            for (w_sb, b_row, dst) in ((wq_sb, bq_row, qt),
                                       (wk_sb, bk_row, kt)):
                for sc in range(4):
                    ps = ppsum.tile([128, 512], F32, tag="pp", name="p1ps")
                    for dc in range(4):
                        nc.tensor.matmul(
                            ps[:],
                            w_sb[dc][:, 128 * p:128 * (p + 1)],
                            xT_sb[dc][:, 512 * sc:512 * (sc + 1)],
                            start=(dc == 0), stop=False)
                    nc.tensor.matmul(
                        ps[:],
                        b_row[:, 128 * p:128 * (p + 1)],
                        ones_row[:, 512 * sc:512 * (sc + 1)],
                        start=False, stop=True)
                    nc.vector.tensor_copy(
                        dst[:, 512 * sc:512 * (sc + 1)], ps[:])

        # ---------- P1b: V augmented, natural layout (bf16) ----------
        vaug = pers.tile([128, 16 * VWS], BF16, tag="vaug", name="vaug")
        for st in range(16):
            ps = ppsum.tile([128, VWS], F32, tag="pp", name="p1vps")
            for dc in range(4):
                nc.tensor.matmul(
                    ps[:],
                    xT_sb[dc][:, 128 * st:128 * (st + 1)],
                    wva_sb[dc][:],
                    start=(dc == 0), stop=False)
            nc.tensor.matmul(
                ps[:],
                ones_row[:, 128 * st:128 * (st + 1)],
                wva_last[:],
                start=False, stop=True)
            nc.vector.tensor_copy(vaug[:, VWS * st:VWS * (st + 1)], ps[:])

        # ---------- P2: attention per head ----------
        OT, OTN = [], []
        for p in range(2):
            ot = pers.tile([128, S], F32, tag=f"OT{p}", name=f"OT{p}")
            OT.append(ot)
            otn = pers.tile([128, S], BF16, tag=f"OTN{p}", name=f"OTN{p}")
            OTN.append(otn)
        # denominator gather target: head-pair p on partitions 64p..64p+64;
        # within a pair: partition 64p + (4*sub+qq)*8 + j, free f holds
        # denom[q = 512*qq + 64*j + f] of head 2p+sub
        d128 = pers.tile([128, 64], F32, tag="d128", name="d128")
        rall = []
        for p in range(2):
            r = pers.tile([1, 4096], BF16, tag=f"rall{p}", name=f"rall{p}")
            rall.append(r)

        # memset the scores psum buffer once (exp may read lanes no matmul
        # wrote this iteration; stale-but-bounded is fine, uninit is not)
        ps_s_init = spsum.tile([128, 2048], F32, tag="ps_s", name="ps_s")
        nc.vector.memset(ps_s_init[:], 0.0)

        for h in range(HPC):
            p, sub = h // 2, h % 2
            qrows = slice(64 * sub, 64 * sub + 64)
            for qq in range(4):
                ps_o = opsum.tile([VW, 512], F32, tag="ps_o", name="ps_o")
                for g in range(qq + 1):
                    ps_s = spsum.tile([128, 2048], F32, tag="ps_s",
                                      name="ps_s")
                    for j in range(4):
                        kk = 4 * g + j
                        so = max(kk * 128 - qq * 512, 0)
                        diag = (g == qq)
                        nc.tensor.matmul(
                            ps_s[:, 512 * j + so:512 * (j + 1)],
                            KT[p][qrows, 128 * kk:128 * (kk + 1)],
                            QT[p][qrows, 512 * qq + so:512 * (qq + 1)],
                            start=True, stop=not diag)
                        if diag:
                            # += -1e5 where k > q on the 128-wide diag block
                            nc.tensor.matmul(
                                ps_s[:, 512 * j + so:512 * j + so + 128],
                                mm_su, mm_ni,
                                start=False, stop=True,
                                skip_group_check=True)
                    pt = ptp.tile([128, 2048], BF16, tag="pt", name="pt")
                    nc.scalar.activation(pt[:], ps_s[:], AF.Exp, scale=SCALE)
                    for j in range(4):
                        kk = 4 * g + j
                        so = max(kk * 128 - qq * 512, 0)
                        nc.tensor.matmul(
                            ps_o[:, so:512],
                            vaug[:, VWS * kk + VW * h:VWS * kk + VW * h + VW],
                            pt[:, 512 * j + so:512 * (j + 1)],
                            start=(kk == 0), stop=(kk == 4 * qq + 3))
                nc.vector.tensor_copy(
                    OT[p][qrows, 512 * qq:512 * (qq + 1)], ps_o[0:64, :])
                dslot = dnp.tile([1, 512], F32, tag="ds", name="dslot")
                nc.vector.tensor_copy(dslot[:], ps_o[64:65, :])
                base = 64 * p + (4 * sub + qq) * 8
                nc.sync.dma_start(d128[base:base + 8, :], dslot[:])

            if sub == 1:
                # head pair complete: one wide reciprocal + row restore
                r128 = dnp.tile([64, 64], BF16, tag="r128", name=f"r128_{p}")
                with nc.allow_low_precision(reason="bf16 softmax recip"):
                    nc.vector.reciprocal(r128[:], d128[64 * p:64 * p + 64, :])
                nc.sync.dma_start(rall[p][:], r128[:])

        # ---------- P3: normalize + output projection ----------
        for h in range(HPC):
            p, sub = h // 2, h % 2
            qrows = slice(64 * sub, 64 * sub + 64)
            for qq in range(4):
                ps_b = ppsum.tile([64, 512], F32, tag="pp", name="p3bps")
                nc.tensor.matmul(
                    ps_b[:],
                    ones64,
                    rall[p][0:1, (4 * sub + qq) * 512:
                            (4 * sub + qq) * 512 + 512],
                    start=True, stop=True)
                nc.vector.tensor_mul(
                    OTN[p][qrows, 512 * qq:512 * (qq + 1)],
                    OT[p][qrows, 512 * qq:512 * (qq + 1)],
                    ps_b[:])

        for gidx in range(2):
            osb = outp.tile([128, 8 * D], F32, tag="osb", name="osb")
            for u in range(8):
                t = 8 * gidx + u
                ps_f = ppsum.tile([128, 512], F32, tag="pp", name="p3fps")
                for p in range(2):
                    nc.tensor.matmul(
                        ps_f[:],
                        OTN[p][:, 128 * t:128 * (t + 1)],
                        wo_sb[p][:],
                        start=(p == 0), stop=(p == 1))
                nc.vector.tensor_copy(osb[:, D * u:D * (u + 1)], ps_f[:])
            out_view = out[1024 * gidx:1024 * (gidx + 1), :].rearrange(
                "(u p) c -> p u c", p=128)
            nc.sync.dma_start(out_view, osb[:].rearrange(
                "p (u c) -> p u c", u=8))


def make_in_maps(x, w_qkv, b_qkv, w_out, b_out):
    x = np.asarray(x, dtype=np.float32)
    w_qkv = np.asarray(w_qkv, dtype=np.float32)
    b_qkv = np.asarray(b_qkv, dtype=np.float32)
    w_out = np.asarray(w_out, dtype=np.float32)

    wrr = w_qkv.reshape(D, 3, 8, HD)
    br = b_qkv.reshape(3, 8, HD)

    in_maps = []
    for c in range(NCORES):
        b = c // 2
        h0 = 4 * (c % 2)
        xT = np.ascontiguousarray(x[b].T)                       # (512, 2048)
        wq = wrr[:, 0, h0:h0 + 4].reshape(D, 256)
        wk = wrr[:, 1, h0:h0 + 4].reshape(D, 256)
        wv = wrr[:, 2, h0:h0 + 4].reshape(D, 256)
        bv = br[2, h0:h0 + 4].reshape(256)
        wva = np.zeros((D + 1, VWS), dtype=np.float32)
        for j in range(HPC):
            wva[:D, VW * j:VW * j + HD] = wv[:, HD * j:HD * (j + 1)]
            wva[D, VW * j:VW * j + HD] = bv[HD * j:HD * (j + 1)]
            wva[D, VW * j + HD] = 1.0
        bq = br[0, h0:h0 + 4].reshape(256)
        bk = br[1, h0:h0 + 4].reshape(256)
        wo = w_out.reshape(8, HD, D)[h0:h0 + 4].reshape(256, D)

        wpack = np.zeros((128, FTOT), dtype=np.float32)
        for dc in range(4):
            wpack[:, OFF_XT + S * dc:OFF_XT + S * (dc + 1)] = \
                xT[128 * dc:128 * (dc + 1)]
            wpack[:, OFF_WQ + 256 * dc:OFF_WQ + 256 * (dc + 1)] = \
                wq[128 * dc:128 * (dc + 1)]
            wpack[:, OFF_WK + 256 * dc:OFF_WK + 256 * (dc + 1)] = \
                wk[128 * dc:128 * (dc + 1)]
            wpack[:, OFF_WVA + VWS * dc:OFF_WVA + VWS * (dc + 1)] = \
                wva[128 * dc:128 * (dc + 1)]
        wpack[0, OFF_WVL:OFF_WVL + VWS] = wva[D]
        for p in range(2):
            wpack[:, OFF_WO + D * p:OFF_WO + D * (p + 1)] = \
                wo[128 * p:128 * (p + 1)]
        wpack[0, OFF_BQ:OFF_BQ + 256] = bq
        wpack[0, OFF_BK:OFF_BK + 256] = bk
        wpack[0, OFF_ONES:OFF_ONES + S] = 1.0
        wpack[:, OFF_SU:OFF_SU + 128] = np.triu(np.ones((128, 128)), k=1)
        wpack[:, OFF_NI:OFF_NI + 128] = np.eye(128) * -1e5

        in_maps.append({"wpack": wpack.astype(ml_dtypes.bfloat16)})
    return in_maps


_NC_CACHE = None


def get_nc():
    global _NC_CACHE
    if _NC_CACHE is None:
        _NC_CACHE = build_nc()
    return _NC_CACHE


def run_cores(x, w_qkv, b_qkv, w_out, b_out, trace=False, trace_cores=None):
    nc = get_nc()
    in_maps = make_in_maps(x, w_qkv, b_qkv, w_out, b_out)
    br = run_bass_kernel_spmd(
        nc, in_maps, list(range(NCORES)),
        trace=trace, trace_cores=trace_cores)
    return br


def assemble(results, b_out):
    b_out = np.asarray(b_out, dtype=np.float32)
    out = np.empty((4, S, D), dtype=np.float32)
    for b in range(4):
        out[b] = results[2 * b]["out"] + results[2 * b + 1]["out"] + b_out
    return out


def kernel(x, w_qkv, b_qkv, w_out, b_out):
    br = run_cores(x, w_qkv, b_qkv, w_out, b_out, trace=False)
    return assemble(br.results, b_out)


# revision 26
# speedup vs baseline: 2.0434x; 1.6502x over previous
"""Causal multi-head attention kernel for 8 Trainium2 NeuronCores.

Problem: x(4,2048,512) -> qkv proj -> 8-head causal attention -> out proj.
Sharding: core c handles batch b=c//2, heads 4*(c%2)..4*(c%2)+3.
Each core returns a partial (2048,512) output (its 4 heads' contribution
through w_out); host sums the two cores of each batch and adds b_out.

Per-core device algorithm (bf16 matmuls, fp32 psum/softmax):
  P1  QKV^T projection from host-pretransposed xT (512,2048):
        QT/KT stored (128=2heads, 2048) per pair; V stored natural with a
        ones column (from an augmented weight matrix with a bias row) so
        the PV matmul also produces softmax denominators. Biases fold in
        as rank-1 (bias row x ones row) matmuls.
  P2  per head: S_T = K Q^T (k on partitions, q free) in fp32 PSUM,
        causal diag masked by accumulating -1e5*(k>q) via a bf16 matmul,
        exp via ACT (scale=1/8 folded, no max subtraction -- scores are
        O(7)), then out'/denom = [V|1]^T @ P~^T accumulated in PSUM.
  P3  denominators DMA-gathered into a 128-partition tile (DVE
        reciprocal is 8 cyc/elem/lane -- needs all lanes), one
        reciprocal per head-pair, DMA back to a row, broadcast across
        partitions via a ones(1,64) matmul, normalize out', project.
"""

import os
import sys

import numpy as np

if "/opt/trn_rl_repo" not in sys.path:
    sys.path.insert(0, "/opt/trn_rl_repo")

import ml_dtypes

import concourse.bass as bass
import concourse.mybir as mybir
import concourse.tile as tile
from concourse import bacc
from concourse.bass_utils import run_bass_kernel_spmd

F32 = mybir.dt.float32
BF16 = mybir.dt.bfloat16
AF = mybir.ActivationFunctionType

S = 2048
D = 512
HD = 64
HPC = 4          # heads per core
NCORES = 8
SCALE = 0.125    # 1/sqrt(64)
VW = HD + 1      # 65: V plus ones column
VWS = HPC * VW   # 260

# column offsets inside the packed bf16 (128, FTOT) input
OFF_XT = 0                      # 4 tiles of (128, 2048)
OFF_WQ = OFF_XT + 4 * S         # 4 tiles of (128, 256)
OFF_WK = OFF_WQ + 4 * 256
OFF_WVA = OFF_WK + 4 * 256      # 4 tiles of (128, 260)
OFF_WVL = OFF_WVA + 4 * VWS     # (128, 260), row 0 = bias/ones row
OFF_WO = OFF_WVL + VWS          # 2 tiles of (128, 512)
OFF_BQ = OFF_WO + 2 * D         # row 0: bq (256)
OFF_BK = OFF_BQ + 256           # row 0: bk (256)
OFF_ONES = OFF_BK + 256         # row 0: ones (2048)
OFF_SU = OFF_ONES + S           # (128,128) strict-upper ones (diag mask)
OFF_NI = OFF_SU + 128           # (128,128) -1e5 * I
FTOT = OFF_NI + 128


def build_nc():
    nc = bacc.Bacc("TRN2", target_bir_lowering=False, debug=False)

    wpack = nc.dram_tensor("wpack", [128, FTOT], BF16,
                           kind="ExternalInput").ap()
    out = nc.dram_tensor("out", [S, D], F32, kind="ExternalOutput").ap()

    with tile.TileContext(nc) as tc:
        _build_kernel(tc, wpack, out)
    nc.compile()
    return nc


def _build_kernel(tc, wpack, out):
    nc = tc.nc
    from contextlib import ExitStack

    ctx = ExitStack()
    with ctx:
        pers = ctx.enter_context(tc.tile_pool(name="pers", bufs=1))
        ppsum = ctx.enter_context(
            tc.tile_pool(name="ppsum", bufs=2, space="PSUM"))   # P1/P3
        spsum = ctx.enter_context(
            tc.tile_pool(name="spsum", bufs=2, space="PSUM"))   # scores
        opsum = ctx.enter_context(
            tc.tile_pool(name="opsum", bufs=2, space="PSUM"))   # PV accum
        ptp = ctx.enter_context(tc.tile_pool(name="ptp", bufs=4))
        outp = ctx.enter_context(tc.tile_pool(name="outp", bufs=2))
        dnp = ctx.enter_context(tc.tile_pool(name="dnp", bufs=3))

        # ---------- P0: one packed bf16 DMA ----------
        wr = pers.tile([128, FTOT], BF16, tag="wr", name="wr")
        nc.sync.dma_start(wr[:], wpack[:])

        xT_sb = [wr[:, OFF_XT + S * dc:OFF_XT + S * (dc + 1)]
                 for dc in range(4)]
        wq_sb = [wr[:, OFF_WQ + 256 * dc:OFF_WQ + 256 * (dc + 1)]
                 for dc in range(4)]
        wk_sb = [wr[:, OFF_WK + 256 * dc:OFF_WK + 256 * (dc + 1)]
                 for dc in range(4)]
        wva_sb = [wr[:, OFF_WVA + VWS * dc:OFF_WVA + VWS * (dc + 1)]
                  for dc in range(4)]
        wva_last = wr[0:1, OFF_WVL:OFF_WVL + VWS]
        wo_sb = [wr[:, OFF_WO + D * p:OFF_WO + D * (p + 1)]
                 for p in range(2)]
        bq_row = wr[0:1, OFF_BQ:OFF_BQ + 256]
        bk_row = wr[0:1, OFF_BK:OFF_BK + 256]
        ones_row = wr[0:1, OFF_ONES:OFF_ONES + S]
        ones64 = wr[0:1, OFF_ONES:OFF_ONES + 64]
        mm_su = wr[:, OFF_SU:OFF_SU + 128]
        mm_ni = wr[:, OFF_NI:OFF_NI + 128]

        # ---------- P1a: QT / KT per pair (bf16) ----------
        QT, KT = [], []
        for p in range(2):
            qt = pers.tile([128, S], BF16, tag=f"QT{p}", name=f"QT{p}")
            kt = pers.tile([128, S], BF16, tag=f"KT{p}", name=f"KT{p}")
            QT.append(qt)
            KT.append(kt)
            for (w_sb, b_row, dst) in ((wq_sb, bq_row, qt),
                                       (wk_sb, bk_row, kt)):
                for sc in range(4):
                    ps = ppsum.tile([128, 512], F32, tag="pp", name="p1ps")
                    for dc in range(4):
                        nc.tensor.matmul(
                            ps[:],
                            w_sb[dc][:, 128 * p:128 * (p + 1)],
                            xT_sb[dc][:, 512 * sc:512 * (sc + 1)],
                            start=(dc == 0), stop=False)
                    nc.tensor.matmul(
                        ps[:],
                        b_row[:, 128 * p:128 * (p + 1)],
                        ones_row[:, 512 * sc:512 * (sc + 1)],
                        start=False, stop=True)
                    nc.vector.tensor_copy(
                        dst[:, 512 * sc:512 * (sc + 1)], ps[:])

        # ---------- P1b: V augmented, natural layout (bf16) ----------
        vaug = pers.tile([128, 16 * VWS], BF16, tag="vaug", name="vaug")
        for st in range(16):
            ps = ppsum.tile([128, VWS], F32, tag="pp", name="p1vps")
            for dc in range(4):
                nc.tensor.matmul(
                    ps[:],
                    xT_sb[dc][:, 128 * st:128 * (st + 1)],
                    wva_sb[dc][:],
                    start=(dc == 0), stop=False)
            nc.tensor.matmul(
                ps[:],
                ones_row[:, 128 * st:128 * (st + 1)],
                wva_last[:],
                start=False, stop=True)
            nc.vector.tensor_copy(vaug[:, VWS * st:VWS * (st + 1)], ps[:])

        # ---------- P2: attention per head ----------
        OT, OTN = [], []
        for p in range(2):
            ot = pers.tile([128, S], F32, tag=f"OT{p}", name=f"OT{p}")
            OT.append(ot)
            otn = pers.tile([128, S], BF16, tag=f"OTN{p}", name=f"OTN{p}")
            OTN.append(otn)
        # denominator gather target: head-pair p on partitions 64p..64p+64;
        # within a pair: partition 64p + (4*sub+qq)*8 + j, free f holds
        # denom[q = 512*qq + 64*j + f] of head 2p+sub
        d128 = pers.tile([128, 64], F32, tag="d128", name="d128")
        rall = []
        for p in range(2):
            r = pers.tile([1, 4096], BF16, tag=f"rall{p}", name=f"rall{p}")
            rall.append(r)

        # memset the scores psum buffers once (exp may read lanes no matmul
        # wrote this iteration; stale-but-bounded is fine, uninit is not)
        for _ in range(2):
            ps_s_init = spsum.tile([128, 1024], F32, tag="ps_s", name="ps_s")
            nc.vector.memset(ps_s_init[:], 0.0)

        for h in range(HPC):
            p, sub = h // 2, h % 2
            qrows = slice(64 * sub, 64 * sub + 64)
            for qq in range(4):
                ps_o = opsum.tile([VW, 512], F32, tag="ps_o", name="ps_o")
                for g in range(2 * (qq + 1)):
                    ps_s = spsum.tile([128, 1024], F32, tag="ps_s",
                                      name="ps_s")
                    diag = (g >= 2 * qq)
                    for j in range(2):
                        kk = 2 * g + j
                        so = max(kk * 128 - qq * 512, 0)
                        nc.tensor.matmul(
                            ps_s[:, 512 * j + so:512 * (j + 1)],
                            KT[p][qrows, 128 * kk:128 * (kk + 1)],
                            QT[p][qrows, 512 * qq + so:512 * (qq + 1)],
                            start=True, stop=not diag)
                        if diag:
                            # += -1e5 where k > q on the 128-wide diag block
                            nc.tensor.matmul(
                                ps_s[:, 512 * j + so:512 * j + so + 128],
                                mm_su, mm_ni,
                                start=False, stop=True,
                                skip_group_check=True)
                    pt = ptp.tile([128, 1024], BF16, tag="pt", name="pt")
                    nc.scalar.activation(pt[:], ps_s[:], AF.Exp, scale=SCALE)
                    for j in range(2):
                        kk = 2 * g + j
                        so = max(kk * 128 - qq * 512, 0)
                        nc.tensor.matmul(
                            ps_o[:, so:512],
                            vaug[:, VWS * kk + VW * h:VWS * kk + VW * h + VW],
                            pt[:, 512 * j + so:512 * (j + 1)],
                            start=(kk == 0), stop=(kk == 4 * qq + 3))
                nc.vector.tensor_copy(
                    OT[p][qrows, 512 * qq:512 * (qq + 1)], ps_o[0:64, :])
                dslot = dnp.tile([1, 512], F32, tag="ds", name="dslot")
                nc.vector.tensor_copy(dslot[:], ps_o[64:65, :])
                base = 64 * p + (4 * sub + qq) * 8
                nc.sync.dma_start(d128[base:base + 8, :], dslot[:])

            if sub == 1:
                # head pair complete: one wide reciprocal + row restore
                r128 = dnp.tile([64, 64], BF16, tag="r128", name=f"r128_{p}")
                with nc.allow_low_precision(reason="bf16 softmax recip"):
                    nc.vector.reciprocal(r128[:], d128[64 * p:64 * p + 64, :])
                nc.sync.dma_start(rall[p][:], r128[:])

        # ---------- P3: normalize + output projection ----------
        for h in range(HPC):
            p, sub = h // 2, h % 2
            qrows = slice(64 * sub, 64 * sub + 64)
            for qq in range(4):
                ps_b = ppsum.tile([64, 512], F32, tag="pp", name="p3bps")
                nc.tensor.matmul(
                    ps_b[:],
                    ones64,
                    rall[p][0:1, (4 * sub + qq) * 512:
                            (4 * sub + qq) * 512 + 512],
                    start=True, stop=True)
                nc.vector.tensor_mul(
                    OTN[p][qrows, 512 * qq:512 * (qq + 1)],
                    OT[p][qrows, 512 * qq:512 * (qq + 1)],
                    ps_b[:])

        for gidx in range(2):
            osb = outp.tile([128, 8 * D], F32, tag="osb", name="osb")
            for u in range(8):
                t = 8 * gidx + u
                ps_f = ppsum.tile([128, 512], F32, tag="pp", name="p3fps")
                for p in range(2):
                    nc.tensor.matmul(
                        ps_f[:],
                        OTN[p][:, 128 * t:128 * (t + 1)],
                        wo_sb[p][:],
                        start=(p == 0), stop=(p == 1))
                nc.vector.tensor_copy(osb[:, D * u:D * (u + 1)], ps_f[:])
            out_view = out[1024 * gidx:1024 * (gidx + 1), :].rearrange(
                "(u p) c -> p u c", p=128)
            nc.sync.dma_start(out_view, osb[:].rearrange(
                "p (u c) -> p u c", u=8))


def make_in_maps(x, w_qkv, b_qkv, w_out, b_out):
    x = np.asarray(x, dtype=np.float32)
    w_qkv = np.asarray(w_qkv, dtype=np.float32)
    b_qkv = np.asarray(b_qkv, dtype=np.float32)
    w_out = np.asarray(w_out, dtype=np.float32)

    wrr = w_qkv.reshape(D, 3, 8, HD)
    br = b_qkv.reshape(3, 8, HD)

    in_maps = []
    for c in range(NCORES):
        b = c // 2
        h0 = 4 * (c % 2)
        xT = np.ascontiguousarray(x[b].T)                       # (512, 2048)
        wq = wrr[:, 0, h0:h0 + 4].reshape(D, 256)
        wk = wrr[:, 1, h0:h0 + 4].reshape(D, 256)
        wv = wrr[:, 2, h0:h0 + 4].reshape(D, 256)
        bv = br[2, h0:h0 + 4].reshape(256)
        wva = np.zeros((D + 1, VWS), dtype=np.float32)
        for j in range(HPC):
            wva[:D, VW * j:VW * j + HD] = wv[:, HD * j:HD * (j + 1)]
            wva[D, VW * j:VW * j + HD] = bv[HD * j:HD * (j + 1)]
            wva[D, VW * j + HD] = 1.0
        bq = br[0, h0:h0 + 4].reshape(256)
        bk = br[1, h0:h0 + 4].reshape(256)
        wo = w_out.reshape(8, HD, D)[h0:h0 + 4].reshape(256, D)

        wpack = np.zeros((128, FTOT), dtype=np.float32)
        for dc in range(4):
            wpack[:, OFF_XT + S * dc:OFF_XT + S * (dc + 1)] = \
                xT[128 * dc:128 * (dc + 1)]
            wpack[:, OFF_WQ + 256 * dc:OFF_WQ + 256 * (dc + 1)] = \
                wq[128 * dc:128 * (dc + 1)]
            wpack[:, OFF_WK + 256 * dc:OFF_WK + 256 * (dc + 1)] = \
                wk[128 * dc:128 * (dc + 1)]
            wpack[:, OFF_WVA + VWS * dc:OFF_WVA + VWS * (dc + 1)] = \
                wva[128 * dc:128 * (dc + 1)]
        wpack[0, OFF_WVL:OFF_WVL + VWS] = wva[D]
        for p in range(2):
            wpack[:, OFF_WO + D * p:OFF_WO + D * (p + 1)] = \
                wo[128 * p:128 * (p + 1)]
        wpack[0, OFF_BQ:OFF_BQ + 256] = bq
        wpack[0, OFF_BK:OFF_BK + 256] = bk
        wpack[0, OFF_ONES:OFF_ONES + S] = 1.0
        wpack[:, OFF_SU:OFF_SU + 128] = np.triu(np.ones((128, 128)), k=1)
        wpack[:, OFF_NI:OFF_NI + 128] = np.eye(128) * -1e5

        in_maps.append({"wpack": wpack.astype(ml_dtypes.bfloat16)})
    return in_maps


_NC_CACHE = None


def get_nc():
    global _NC_CACHE
    if _NC_CACHE is None:
        _NC_CACHE = build_nc()
    return _NC_CACHE


def run_cores(x, w_qkv, b_qkv, w_out, b_out, trace=False, trace_cores=None):
    nc = get_nc()
    in_maps = make_in_maps(x, w_qkv, b_qkv, w_out, b_out)
    br = run_bass_kernel_spmd(
        nc, in_maps, list(range(NCORES)),
        trace=trace, trace_cores=trace_cores)
    return br


def assemble(results, b_out):
    b_out = np.asarray(b_out, dtype=np.float32)
    out = np.empty((4, S, D), dtype=np.float32)
    for b in range(4):
        out[b] = results[2 * b]["out"] + results[2 * b + 1]["out"] + b_out
    return out


def kernel(x, w_qkv, b_qkv, w_out, b_out):
    br = run_cores(x, w_qkv, b_qkv, w_out, b_out, trace=False)
    return assemble(br.results, b_out)
